# revision 28
# baseline (speedup 1.0000x reference)
"""AnoNAViLa forward kernel for 8 TRN2 NeuronCores (data-parallel over batch).

Math (per branch):
  sims = (img @ text.T) * scale;  w = softmax(sims);  e = exp(w)
  x = concat([img_rep, text * e[..., None]], -1)
  h = relu(x @ W1 + b1); h = relu(h @ W2 + b2); h = h @ W3 + b3
  out = h.mean(axis=1)

Key algebraic restructuring (exact, up to fp assoc):
  x @ W1 = img @ W1[:D] + e[b,n] * (text @ W1[D:])      (rank-1 per (b,n))
  mean_n (h2 @ W3 + b3) = (mean_n h2) @ W3 + b3          (mean before layer 3)
"""
import sys

sys.path.insert(0, "/opt/trn_rl_repo")

from contextlib import ExitStack

import numpy as np
import orjson

import concourse.bass as bass
import concourse.mybir as mybir
import concourse.tile as tile
from concourse.bass import ds, ts
from concourse.bass_utils import run_bass_kernel_spmd
from concourse.masks import make_identity

F32 = mybir.dt.float32
F32R = mybir.dt.float32r
BF16 = mybir.dt.bfloat16
AF = mybir.ActivationFunctionType
ALU = mybir.AluOpType

NC = 8
B, N, D = 1024, 96, 512
BL = B // NC  # 128 rows per core


# ---------------------------------------------------------------------------
# This walrus build rejects instructions with >1 semaphore wait/update
# ("Too many sync wait commands").  Split extras onto chained NoOps on the
# same engine (streams are in-order, so sequential waits == combined wait).
_bir_patch_installed = False


def _split_multi_sync(bir_json: bytes) -> bytes:
    d = orjson.loads(bir_json)
    ctr = [0]

    def mk_nop(inst, wait=None, update=None):
        ctr[0] += 1
        return {
            "debug": inst.get("debug", 0),
            "engine": inst["engine"],
            "ins": [],
            "outs": [],
            "name": f"{inst['name']}__ssplit{ctr[0]}",
            "opcode": "NoOp",
            "sync_info": {
                "on_update": [update] if update else [],
                "on_wait": [wait] if wait else [],
            },
        }

    changed = False
    for fn in d["functions"]:
        for bb in fn["blocks"]:
            new_insts = []
            for inst in bb["instructions"]:
                si = inst.get("sync_info")
                pre, post = [], []
                if si:
                    waits = si.get("on_wait") or []
                    if len(waits) > 1:
                        pre = [mk_nop(inst, wait=w) for w in waits[:-1]]
                        si["on_wait"] = [waits[-1]]
                        changed = True
                    upds = si.get("on_update") or []
                    if len(upds) > 1:
                        post = [mk_nop(inst, update=u) for u in upds[1:]]
                        si["on_update"] = [upds[0]]
                        changed = True
                new_insts.extend(pre)
                new_insts.append(inst)
                new_insts.extend(post)
            bb["instructions"] = new_insts
    return orjson.dumps(d) if changed else bir_json


def _install_bir_patch():
    global _bir_patch_installed
    if _bir_patch_installed:
        return
    _bir_patch_installed = True
    import concourse.bass_utils as bu
    import concourse.bass2jax as b2j

    orig = bu.compile_bir_kernel

    def patched(bir_json, tmpdir, neff_name="file.neff"):
        return orig(_split_multi_sync(bir_json), tmpdir, neff_name)

    bu.compile_bir_kernel = patched
    b2j.compile_bir_kernel = patched



# ---------------------------------------------------------------------------
def build_graph(scale: float, has_b2: bool = True) -> bass.Bass:
    nc = bass.Bass()

    img_ext = nc.declare_dram_parameter("img", [BL, D], F32, isOutput=False)
    tn_ext = nc.declare_dram_parameter("tn", [N, D], F32, isOutput=False)
    ta_ext = nc.declare_dram_parameter("ta", [N, D], F32, isOutput=False)
    W1_ext = nc.declare_dram_parameter("W1", [2 * D, D], F32, isOutput=False)
    b1_ext = nc.declare_dram_parameter("b1", [D], F32, isOutput=False)
    W2_ext = nc.declare_dram_parameter("W2", [D, D // 2], F32, isOutput=False)
    b2_ext = nc.declare_dram_parameter("b2", [D // 2], F32, isOutput=False)
    W3_ext = nc.declare_dram_parameter("W3", [D // 2, D // 4], F32, isOutput=False)
    b3_ext = nc.declare_dram_parameter("b3", [D // 4], F32, isOutput=False)
    idf_ext = nc.declare_dram_parameter("idf", [128, 128], F32, isOutput=False)
    idb_ext = nc.declare_dram_parameter("idb", [128, 128], BF16, isOutput=False)
    zer_ext = nc.declare_dram_parameter("zer", [(N // 4) * 2048], BF16, isOutput=False)
    out_ext = nc.declare_dram_parameter("out", [2, D // 4, BL], F32, isOutput=True)
    # DRAM bounce buffers for the cross-partition flatten of eT / tW
    eT_dram = nc.dram_tensor("eT_dram", [2, N * BL], BF16)
    tW_dram = nc.dram_tensor("tW_dram", [2, N * D], BF16)
    # block-diagonal selector: E4_dram[br, g] is [4, 512] row-major with
    # E4[k, k*128:(k+1)*128] = eT[4g+k, :] and zeros elsewhere
    E4_dram = nc.dram_tensor("E4_dram", [2, N // 4, 4 * 512], BF16)

    with tile.TileContext(nc) as tc, ExitStack() as ctx:
        const = ctx.enter_context(tc.tile_pool(name="const", bufs=1))
        work = ctx.enter_context(tc.tile_pool(name="work", bufs=2))
        hpool = ctx.enter_context(tc.tile_pool(name="hpool", bufs=4))
        psH1 = ctx.enter_context(tc.tile_pool(name="psH1", bufs=2, space="PSUM"))
        psH2 = ctx.enter_context(tc.tile_pool(name="psH2", bufs=2, space="PSUM"))

        # ---- constant loads -------------------------------------------------
        img_s = const.tile([BL, D], F32)
        nc.sync.dma_start(out=img_s[:], in_=img_ext[:, :])
        tn_s = const.tile([N, D], F32)
        nc.sync.dma_start(out=tn_s[:], in_=tn_ext[:, :])
        ta_s = const.tile([N, D], F32)
        nc.sync.dma_start(out=ta_s[:], in_=ta_ext[:, :])
        W1_s = const.tile([128, 8, D], F32)  # W1_s[p, c, :] = W1[c*128+p, :]
        nc.sync.dma_start(out=W1_s[:], in_=W1_ext[:, :].rearrange("(c p) d -> p c d", p=128))
        W2_s = const.tile([128, 4, D // 2], F32)
        nc.sync.dma_start(out=W2_s[:], in_=W2_ext[:, :].rearrange("(c p) d -> p c d", p=128))
        W2_r = const.tile([128, 4, D // 2], F32R)
        nc.vector.tensor_copy(W2_r[:], W2_s[:])
        W1b_bf = const.tile([128, 4, D], BF16)
        nc.scalar.copy(W1b_bf[:], W1_s[:, 4:8, :])
        W3_s = const.tile([128, 2, D // 4], F32)
        nc.sync.dma_start(out=W3_s[:], in_=W3_ext[:, :].rearrange("(c p) d -> p c d", p=128))
        b1t = const.tile([128, 4], F32)
        nc.sync.dma_start(out=b1t[:], in_=b1_ext[:].rearrange("(c p) -> p c", p=128))
        # b2 as a single-partition row (for a rank-1 bias matmul into PSUM)
        b2row = const.tile([1, D // 2], F32)
        nc.sync.dma_start(out=b2row[:], in_=b2_ext[None, :])
        b2row_r = const.tile([1, D // 2], F32R)
        nc.vector.tensor_copy(b2row_r[:], b2row[:])
        ones_f = const.tile([1, 384], F32)
        nc.gpsimd.memset(ones_f[:], 1.0)
        ones_r = const.tile([1, 384], F32R)
        nc.vector.tensor_copy(ones_r[:], ones_f[:])
        b3t = const.tile([128, 1], F32)
        nc.sync.dma_start(out=b3t[:], in_=b3_ext[:].rearrange("(c p) -> p c", p=128))

        I_s = const.tile([128, 128], F32)
        nc.sync.dma_start(out=I_s[:], in_=idf_ext[:, :])
        I_bf = const.tile([128, 128], BF16)
        nc.sync.dma_start(out=I_bf[:], in_=idb_ext[:, :])

        for br0 in range(2):
            nc.sync.dma_start(out=E4_dram[br0, :, :], in_=zer_ext[:])
        # K=128 zero-padded selector / t-row tiles (rows 4..127 stay zero so the
        # quad matmuls run as full K=128 -- no PE tiling-mode switches)
        E4p = [const.tile([128, (N // 4) * 512], BF16, tag=f"E4p{i}", name=f"E4p{i}")
               for i in range(2)]
        tW4p = [const.tile([128, (N // 4) * D], BF16, tag=f"tW4p{i}", name=f"tW4p{i}")
                for i in range(2)]
        nc.gpsimd.memset(E4p[0][:], 0.0)
        nc.gpsimd.memset(tW4p[0][:], 0.0)

        # ---- imgT: [d-part, b-free] (4 chunks of 128) -----------------------
        imgT_s = const.tile([128, 4, BL], F32)
        for c in range(4):
            pt = psH1.tile([128, 128], F32, tag="h1p")
            nc.tensor.transpose(pt[:], img_s[:, ts(c, 128)], I_s[:])
            nc.scalar.copy(imgT_s[:, c, :], pt[:])

        # ---- aT3: img@W1_top + b1, transposed, replicated 3x along free ----
        # aT3_s[:, co, s*128:(s+1)*128] = aT chunk co, for each triad slice s
        aT3_s = const.tile([128, 4, 384], F32R)
        for co in range(4):
            pa = psH2.tile([128, 128], F32, tag="h2p")
            for ci in range(4):
                nc.tensor.matmul(
                    pa[:], W1_s[:, ci, ts(co, 128)], imgT_s[:, ci, :],
                    start=(ci == 0), stop=(ci == 3),
                )
            nc.scalar.activation(
                aT3_s[:, co, 0:128], pa[:], AF.Identity, bias=b1t[:, co : co + 1]
            )
            for r in range(1, 3):
                nc.vector.tensor_copy(aT3_s[:, co, ts(r, 128)], aT3_s[:, co, 0:128])

        # ---- per-branch prologues (both first, so branch-1 prologue PE work
        # overlaps branch-0 main-loop DMA latency) -------------------------
        branch_state = []
        for br, text_s in enumerate((tn_s, ta_s)):
            # textT: [d-part, n-free] (f32 for sims, bf16 for the tW matmul)
            textT_s = work.tile([128, 4, N], F32, tag="textT")
            textT_bf = work.tile([128, 4, N], BF16, tag="textT_bf")
            for c in range(4):
                pt = psH1.tile([128, N], F32, tag="h1p")
                nc.tensor.transpose(pt[:], text_s[:, ts(c, 128)], I_s[:N, :N])
                nc.scalar.copy(textT_s[:, c, :], pt[:])
                nc.scalar.copy(textT_bf[:, c, :], pt[:])

            # sims = img @ text.T (scale folded into the exp activation)
            ps_sims = psH2.tile([BL, N], F32, tag="h2p")
            for c in range(4):
                nc.tensor.matmul(
                    ps_sims[:], imgT_s[:, c, :], textT_s[:, c, :],
                    start=(c == 0), stop=(c == 3),
                )
            # softmax over n then e = exp(w), all rowwise
            negmax = work.tile([BL, 1], F32, tag="negmax")
            nc.vector.tensor_reduce(
                negmax[:], ps_sims[:], axis=mybir.AxisListType.X, op=ALU.max,
                negate=True,
            )
            nb = work.tile([BL, 1], F32, tag="nb")
            nc.vector.tensor_scalar_mul(nb[:], negmax[:], float(scale))
            E_s = work.tile([BL, N], F32, tag="E_s")
            ssum = work.tile([BL, 1], F32, tag="ssum")
            nc.scalar.activation(
                E_s[:], ps_sims[:], AF.Exp, bias=nb[:, 0:1], scale=float(scale),
                accum_out=ssum[:, 0:1],
            )
            rr = work.tile([BL, 1], F32, tag="rr")
            nc.vector.reciprocal(rr[:], ssum[:])
            e_x = work.tile([BL, N], F32, tag="e_x")
            nc.scalar.activation(e_x[:], E_s[:], AF.Exp, scale=rr[:, 0:1])

            # eT, padded to 4 row-strips (n -> partition 32*(n//24) + n%24), bf16
            pe_t = psH1.tile([N, BL], F32, tag="h1p")
            nc.tensor.transpose(pe_t[:], e_x[:], I_s[:])
            if br == 1:
                nc.gpsimd.memset(E4p[1][:], 0.0)
                nc.gpsimd.memset(tW4p[1][:], 0.0)
            eT_bf = work.tile([N, BL], BF16, tag="eT_bf")
            nc.scalar.copy(eT_bf[:], pe_t[:])
            # flatten: partition 32*s holds strip s's 32 rows along free
            # eT_f[32s, j*128:(j+1)*128] = eT[32s+j, :]
            eT_f = work.tile([128, 32 * BL], BF16, tag="eT_f")
            nc.sync.dma_start(out=eT_dram[br, :], in_=eT_bf[:])
            for s in range(3):
                nc.sync.dma_start(
                    out=eT_f[ds(32 * s, 1), :],
                    in_=eT_dram[br, ds(s * 32 * BL, 32 * BL)],
                )

            # tW = text @ W1_bot, same padded layout, bf16
            ptw = psH2.tile([N, D], F32, tag="h2p")
            for c in range(4):
                nc.tensor.matmul(
                    ptw[:], textT_bf[:, c, :], W1b_bf[:, c, :],
                    start=(c == 0), stop=(c == 3),
                )
            tW_bf = work.tile([N, D], BF16, tag="tW_bf")
            nc.scalar.copy(tW_bf[:], ptw[:])
            # tW_f[32s, j*512:(j+1)*512] = tW[32s+j, :]
            tW_f = work.tile([128, 32 * D], BF16, tag="tW_f")
            nc.sync.dma_start(out=tW_dram[br, :], in_=tW_bf[:])
            # preload all quads' t-rows into rows 0..3 of the padded tile
            tW4_all = tW4p[br]
            twd = tW_dram[br, 0:1]
            nc.sync.dma_start(
                out=tW4_all[0:4, :],
                in_=bass.AP(tensor=twd.tensor, offset=twd.offset,
                            ap=[[512, 4], [2048, N // 4], [1, 512]]),
            )
            for s in range(3):
                nc.sync.dma_start(
                    out=tW_f[ds(32 * s, 1), :],
                    in_=tW_dram[br, ds(s * 32 * D, 32 * D)],
                )

            # running sum over n of relu(h2) (4 lanes folded at the end)
            macc3 = work.tile([128, 2, 384], F32, tag="macc3")
            nc.vector.memset(macc3[:], 0.0)

            # ---- main loop: 32 triads of n = {g, 32+g, 64+g} ---------------
            for g in range(32):
                h1_s = hpool.tile([128, 4, 384], F32R, tag="h1")
                for c in range(4):
                    ph1 = psH1.tile([128, 384], F32, tag="h1p")
                    # broadcast aT (+b1) into PSUM via f32r identity matmul
                    nc.tensor.matmul(
                        ph1[:], I_r[:], aT3_s[:, c, :],
                        start=True, stop=True,
                    )
                    # rank-1 updates: + t[n, c-chunk] (x) e[:, n], per triad slice
                    for s in range(3):
                        nc.tensor.matmul(
                            ph1[:, ts(s, 128)],
                            tW_f[ds(32 * s, 1), ds(g * D + c * 128, 128)],
                            eT_f[ds(32 * s, 1), ts(g, BL)],
                            start=False, stop=True, skip_group_check=True,
                            tile_position=(32 * s, 0),
                        )
                    # relu -> SBUF (split across ScalarE / VectorE)
                    if c < 3:
                        nc.scalar.activation(h1_s[:, c, :], ph1[:], AF.Relu)
                    else:
                        nc.vector.tensor_scalar_max(h1_s[:, c, :], ph1[:], 0.0)

                for m in range(2):
                    ph2 = psH2.tile([128, 384], F32, tag="h2p")
                    # seed PSUM with b2 broadcast: b2_half (x) ones
                    nc.tensor.matmul(
                        ph2[:],
                        b2row_r[0:1, ts(m, 128)], ones_r[0:1, :],
                        start=True, stop=False,
                    )
                    for c in range(4):
                        nc.tensor.matmul(
                            ph2[:],
                            W2_r[:, c, ts(m, 128)], h1_s[:, c, :],
                            start=False, stop=(c == 3),
                        )
                    # fused: macc3 += relu(ph2)
                    nc.vector.scalar_tensor_tensor(
                        out=macc3[:, m, :], in0=ph2[:], scalar=0.0,
                        in1=macc3[:, m, :], op0=ALU.max, op1=ALU.add,
                    )

            # fold 3 lanes then layer 3
            mfold = work.tile([128, 2, 128], F32, tag="mfold")
            nc.vector.tensor_add(mfold[:], macc3[:, :, 0:128], macc3[:, :, 128:256])
            mred = work.tile([128, 2, 128], F32, tag="mred")
            nc.vector.tensor_add(mred[:], mfold[:], macc3[:, :, 256:384])
            po = psH1.tile([128, 128], F32, tag="h1p")
            for m in range(2):
                nc.tensor.matmul(
                    po[:], W3_s[:, m, :], mred[:, m, :],
                    start=(m == 0), stop=(m == 1),
                )
            outT = work.tile([128, 128], F32, tag="outT")
            nc.vector.tensor_scalar(
                out=outT[:], in0=po[:], scalar1=1.0 / N, scalar2=b3t[:, 0:1],
                op0=ALU.mult, op1=ALU.add,
            )
            nc.sync.dma_start(out=out_ext[br, :, :], in_=outT[:])

    return nc


def make_in_maps(inputs):
    import ml_dtypes

    img = np.ascontiguousarray(np.asarray(inputs["img_embs"], np.float32))
    shared = {
        "tn": np.ascontiguousarray(np.asarray(inputs["normal_text_embs"], np.float32)),
        "ta": np.ascontiguousarray(np.asarray(inputs["abnormal_text_embs"], np.float32)),
        "W1": np.ascontiguousarray(np.asarray(inputs["W1"], np.float32)),
        "b1": np.ascontiguousarray(np.asarray(inputs["b1"], np.float32)),
        "W2": np.ascontiguousarray(np.asarray(inputs["W2"], np.float32)),
        "b2": np.ascontiguousarray(np.asarray(inputs["b2"], np.float32)),
        "W3": np.ascontiguousarray(np.asarray(inputs["W3"], np.float32)),
        "b3": np.ascontiguousarray(np.asarray(inputs["b3"], np.float32)),
        "idf": np.eye(128, dtype=np.float32),
        "idb": np.eye(128).astype(ml_dtypes.bfloat16),
        "zer": np.zeros((N // 4) * 2048, dtype=ml_dtypes.bfloat16),
    }
    return [dict(shared, img=img[i * BL : (i + 1) * BL]) for i in range(NC)]


def kernel(**inputs) -> tuple:
    _install_bir_patch()

    b2 = np.asarray(inputs["b2"], np.float32)
    scale = float(np.exp(np.asarray(inputs["logit_scale"], np.float32).reshape(-1)[0]))

    nc = build_graph(scale, has_b2=bool(np.any(b2)))
    in_maps = make_in_maps(inputs)
    res = run_bass_kernel_spmd(nc, in_maps, core_ids=list(range(NC)))
    h_n = np.concatenate([res.results[i]["out"][0].T for i in range(NC)], axis=0)
    h_a = np.concatenate([res.results[i]["out"][1].T for i in range(NC)], axis=0)
    return (h_n, h_a)


# revision 29
# speedup vs baseline: 1.0783x; 1.0783x over previous
"""AnoNAViLa forward kernel for 8 TRN2 NeuronCores (data-parallel over batch).

Math (per branch):
  sims = (img @ text.T) * scale;  w = softmax(sims);  e = exp(w)
  x = concat([img_rep, text * e[..., None]], -1)
  h = relu(x @ W1 + b1); h = relu(h @ W2 + b2); h = h @ W3 + b3
  out = h.mean(axis=1)

Key algebraic restructuring (exact, up to fp assoc):
  x @ W1 = img @ W1[:D] + e[b,n] * (text @ W1[D:])      (rank-1 per (b,n))
  mean_n (h2 @ W3 + b3) = (mean_n h2) @ W3 + b3          (mean before layer 3)
"""
import sys

sys.path.insert(0, "/opt/trn_rl_repo")

from contextlib import ExitStack

import numpy as np
import orjson

import concourse.bass as bass
import concourse.mybir as mybir
import concourse.tile as tile
from concourse.bass import ds, ts
from concourse.bass_utils import run_bass_kernel_spmd
from concourse.masks import make_identity

F32 = mybir.dt.float32
F32R = mybir.dt.float32r
BF16 = mybir.dt.bfloat16
AF = mybir.ActivationFunctionType
ALU = mybir.AluOpType

NC = 8
B, N, D = 1024, 96, 512
BL = B // NC  # 128 rows per core


# ---------------------------------------------------------------------------
# This walrus build rejects instructions with >1 semaphore wait/update
# ("Too many sync wait commands").  Split extras onto chained NoOps on the
# same engine (streams are in-order, so sequential waits == combined wait).
_bir_patch_installed = False


def _split_multi_sync(bir_json: bytes) -> bytes:
    d = orjson.loads(bir_json)
    ctr = [0]

    def mk_nop(inst, wait=None, update=None):
        ctr[0] += 1
        return {
            "debug": inst.get("debug", 0),
            "engine": inst["engine"],
            "ins": [],
            "outs": [],
            "name": f"{inst['name']}__ssplit{ctr[0]}",
            "opcode": "NoOp",
            "sync_info": {
                "on_update": [update] if update else [],
                "on_wait": [wait] if wait else [],
            },
        }

    changed = False
    for fn in d["functions"]:
        for bb in fn["blocks"]:
            new_insts = []
            for inst in bb["instructions"]:
                si = inst.get("sync_info")
                pre, post = [], []
                if si:
                    waits = si.get("on_wait") or []
                    if len(waits) > 1:
                        pre = [mk_nop(inst, wait=w) for w in waits[:-1]]
                        si["on_wait"] = [waits[-1]]
                        changed = True
                    upds = si.get("on_update") or []
                    if len(upds) > 1:
                        post = [mk_nop(inst, update=u) for u in upds[1:]]
                        si["on_update"] = [upds[0]]
                        changed = True
                new_insts.extend(pre)
                new_insts.append(inst)
                new_insts.extend(post)
            bb["instructions"] = new_insts
    return orjson.dumps(d) if changed else bir_json


def _install_bir_patch():
    global _bir_patch_installed
    if _bir_patch_installed:
        return
    _bir_patch_installed = True
    import concourse.bass_utils as bu
    import concourse.bass2jax as b2j

    orig = bu.compile_bir_kernel

    def patched(bir_json, tmpdir, neff_name="file.neff"):
        return orig(_split_multi_sync(bir_json), tmpdir, neff_name)

    bu.compile_bir_kernel = patched
    b2j.compile_bir_kernel = patched



# ---------------------------------------------------------------------------
def build_graph(scale: float, has_b2: bool = True) -> bass.Bass:
    nc = bass.Bass()

    img_ext = nc.declare_dram_parameter("img", [BL, D], F32, isOutput=False)
    tn_ext = nc.declare_dram_parameter("tn", [N, D], F32, isOutput=False)
    ta_ext = nc.declare_dram_parameter("ta", [N, D], F32, isOutput=False)
    W1_ext = nc.declare_dram_parameter("W1", [2 * D, D], F32, isOutput=False)
    b1_ext = nc.declare_dram_parameter("b1", [D], F32, isOutput=False)
    W2_ext = nc.declare_dram_parameter("W2", [D, D // 2], F32, isOutput=False)
    b2_ext = nc.declare_dram_parameter("b2", [D // 2], F32, isOutput=False)
    W3_ext = nc.declare_dram_parameter("W3", [D // 2, D // 4], F32, isOutput=False)
    b3_ext = nc.declare_dram_parameter("b3", [D // 4], F32, isOutput=False)
    idf_ext = nc.declare_dram_parameter("idf", [128, 128], F32, isOutput=False)
    idb_ext = nc.declare_dram_parameter("idb", [128, 128], BF16, isOutput=False)
    zer_ext = nc.declare_dram_parameter("zer", [(N // 4) * 2048], BF16, isOutput=False)
    out_ext = nc.declare_dram_parameter("out", [2, D // 4, BL], F32, isOutput=True)
    # DRAM bounce buffers for the cross-partition flatten of eT / tW
    eT_dram = nc.dram_tensor("eT_dram", [2, N * BL], BF16)
    tW_dram = nc.dram_tensor("tW_dram", [2, N * D], BF16)
    # block-diagonal selector: E4_dram[br, g] is [4, 512] row-major with
    # E4[k, k*128:(k+1)*128] = eT[4g+k, :] and zeros elsewhere
    E4_dram = nc.dram_tensor("E4_dram", [2, N // 4, 4 * 512], BF16)

    with tile.TileContext(nc) as tc, ExitStack() as ctx:
        const = ctx.enter_context(tc.tile_pool(name="const", bufs=1))
        work = ctx.enter_context(tc.tile_pool(name="work", bufs=2))
        hpool = ctx.enter_context(tc.tile_pool(name="hpool", bufs=3))
        psH1 = ctx.enter_context(tc.tile_pool(name="psH1", bufs=2, space="PSUM"))
        psH2 = ctx.enter_context(tc.tile_pool(name="psH2", bufs=2, space="PSUM"))

        # ---- constant loads -------------------------------------------------
        img_s = const.tile([BL, D], F32)
        nc.sync.dma_start(out=img_s[:], in_=img_ext[:, :])
        tn_s = const.tile([N, D], F32)
        nc.sync.dma_start(out=tn_s[:], in_=tn_ext[:, :])
        ta_s = const.tile([N, D], F32)
        nc.sync.dma_start(out=ta_s[:], in_=ta_ext[:, :])
        W1_s = const.tile([128, 8, D], F32)  # W1_s[p, c, :] = W1[c*128+p, :]
        nc.sync.dma_start(out=W1_s[:], in_=W1_ext[:, :].rearrange("(c p) d -> p c d", p=128))
        W2_s = const.tile([128, 4, D // 2], F32)
        nc.sync.dma_start(out=W2_s[:], in_=W2_ext[:, :].rearrange("(c p) d -> p c d", p=128))
        W2_r = const.tile([128, 4, D // 2], F32R)
        nc.vector.tensor_copy(W2_r[:], W2_s[:])
        W1b_bf = const.tile([128, 4, D], BF16)
        nc.scalar.copy(W1b_bf[:], W1_s[:, 4:8, :])
        W3_s = const.tile([128, 2, D // 4], F32)
        nc.sync.dma_start(out=W3_s[:], in_=W3_ext[:, :].rearrange("(c p) d -> p c d", p=128))
        b1t = const.tile([128, 4], F32)
        nc.sync.dma_start(out=b1t[:], in_=b1_ext[:].rearrange("(c p) -> p c", p=128))
        # b2 as a single-partition row (for a rank-1 bias matmul into PSUM)
        b2row = const.tile([1, D // 2], F32)
        nc.sync.dma_start(out=b2row[:], in_=b2_ext[None, :])
        b2row_r = const.tile([1, D // 2], F32R)
        nc.vector.tensor_copy(b2row_r[:], b2row[:])
        ones_f = const.tile([1, 384], F32)
        nc.gpsimd.memset(ones_f[:], 1.0)
        ones_r = const.tile([1, 384], F32R)
        nc.vector.tensor_copy(ones_r[:], ones_f[:])
        b3t = const.tile([128, 1], F32)
        nc.sync.dma_start(out=b3t[:], in_=b3_ext[:].rearrange("(c p) -> p c", p=128))

        I_s = const.tile([128, 128], F32)
        nc.sync.dma_start(out=I_s[:], in_=idf_ext[:, :])
        I_bf = const.tile([128, 128], BF16)
        nc.sync.dma_start(out=I_bf[:], in_=idb_ext[:, :])

        for br0 in range(2):
            nc.sync.dma_start(out=E4_dram[br0, :, :], in_=zer_ext[:])
        # K=128 zero-padded selector / t-row tiles (rows 4..127 stay zero so the
        # quad matmuls run as full K=128 -- no PE tiling-mode switches)
        E4p = [const.tile([128, (N // 4) * 512], BF16, tag=f"E4p{i}", name=f"E4p{i}")
               for i in range(2)]
        tW4p = [const.tile([128, (N // 4) * D], BF16, tag=f"tW4p{i}", name=f"tW4p{i}")
                for i in range(2)]
        nc.gpsimd.memset(E4p[0][:], 0.0)
        nc.gpsimd.memset(tW4p[0][:], 0.0)

        # ---- imgT: [d-part, b-free] (4 chunks of 128) -----------------------
        imgT_s = const.tile([128, 4, BL], F32)
        for c in range(4):
            pt = psH1.tile([128, 128], F32, tag="h1p")
            nc.tensor.transpose(pt[:], img_s[:, ts(c, 128)], I_s[:])
            nc.scalar.copy(imgT_s[:, c, :], pt[:])

        # ---- aT3: img@W1_top + b1, transposed, replicated 3x along free ----
        # aT3_s[:, co, s*128:(s+1)*128] = aT chunk co, for each triad slice s
        aT3_s = const.tile([128, 4, 384], F32R)
        for co in range(4):
            pa = psH2.tile([128, 128], F32, tag="h2p")
            for ci in range(4):
                nc.tensor.matmul(
                    pa[:], W1_s[:, ci, ts(co, 128)], imgT_s[:, ci, :],
                    start=(ci == 0), stop=(ci == 3),
                )
            nc.scalar.activation(
                aT3_s[:, co, 0:128], pa[:], AF.Identity, bias=b1t[:, co : co + 1]
            )
            for r in range(1, 3):
                nc.vector.tensor_copy(aT3_s[:, co, ts(r, 128)], aT3_s[:, co, 0:128])

        # ---- per-branch prologues (both first, so branch-1 prologue PE work
        # overlaps branch-0 main-loop DMA latency) -------------------------
        branch_state = []
        for br, text_s in enumerate((tn_s, ta_s)):
            # textT: [d-part, n-free] (f32 for sims, bf16 for the tW matmul)
            textT_s = work.tile([128, 4, N], F32, tag="textT")
            textT_bf = work.tile([128, 4, N], BF16, tag="textT_bf")
            for c in range(4):
                pt = psH1.tile([128, N], F32, tag="h1p")
                nc.tensor.transpose(pt[:], text_s[:, ts(c, 128)], I_s[:N, :N])
                nc.scalar.copy(textT_s[:, c, :], pt[:])
                nc.scalar.copy(textT_bf[:, c, :], pt[:])

            # sims = img @ text.T (scale folded into the exp activation)
            ps_sims = psH2.tile([BL, N], F32, tag="h2p")
            for c in range(4):
                nc.tensor.matmul(
                    ps_sims[:], imgT_s[:, c, :], textT_s[:, c, :],
                    start=(c == 0), stop=(c == 3),
                )
            # softmax over n then e = exp(w), all rowwise
            negmax = work.tile([BL, 1], F32, tag="negmax")
            nc.vector.tensor_reduce(
                negmax[:], ps_sims[:], axis=mybir.AxisListType.X, op=ALU.max,
                negate=True,
            )
            nb = work.tile([BL, 1], F32, tag="nb")
            nc.vector.tensor_scalar_mul(nb[:], negmax[:], float(scale))
            E_s = work.tile([BL, N], F32, tag="E_s")
            ssum = work.tile([BL, 1], F32, tag="ssum")
            nc.scalar.activation(
                E_s[:], ps_sims[:], AF.Exp, bias=nb[:, 0:1], scale=float(scale),
                accum_out=ssum[:, 0:1],
            )
            rr = work.tile([BL, 1], F32, tag="rr")
            nc.vector.reciprocal(rr[:], ssum[:])
            e_x = work.tile([BL, N], F32, tag="e_x")
            nc.scalar.activation(e_x[:], E_s[:], AF.Exp, scale=rr[:, 0:1])

            # eT, padded to 4 row-strips (n -> partition 32*(n//24) + n%24), bf16
            pe_t = psH1.tile([N, BL], F32, tag="h1p")
            nc.tensor.transpose(pe_t[:], e_x[:], I_s[:])
            if br == 1:
                nc.gpsimd.memset(E4p[1][:], 0.0)
                nc.gpsimd.memset(tW4p[1][:], 0.0)
            eT_bf = work.tile([N, BL], BF16, tag="eT_bf")
            nc.scalar.copy(eT_bf[:], pe_t[:])
            # flatten: partition 32*s holds strip s's 32 rows along free
            # eT_f[32s, j*128:(j+1)*128] = eT[32s+j, :]
            eT_f = work.tile([128, 32 * BL], BF16, tag="eT_f")
            nc.sync.dma_start(out=eT_dram[br, :], in_=eT_bf[:])
            for s in range(3):
                nc.sync.dma_start(
                    out=eT_f[ds(32 * s, 1), :],
                    in_=eT_dram[br, ds(s * 32 * BL, 32 * BL)],
                )

            # tW = text @ W1_bot, same padded layout, bf16
            ptw = psH2.tile([N, D], F32, tag="h2p")
            for c in range(4):
                nc.tensor.matmul(
                    ptw[:], textT_bf[:, c, :], W1b_bf[:, c, :],
                    start=(c == 0), stop=(c == 3),
                )
            tW_bf = work.tile([N, D], BF16, tag="tW_bf")
            nc.scalar.copy(tW_bf[:], ptw[:])
            # tW_f[32s, j*512:(j+1)*512] = tW[32s+j, :]
            tW_f = work.tile([128, 32 * D], BF16, tag="tW_f")
            nc.sync.dma_start(out=tW_dram[br, :], in_=tW_bf[:])
            # preload all quads' t-rows into rows 0..3 of the padded tile
            tW4_all = tW4p[br]
            twd = tW_dram[br, 0:1]
            nc.sync.dma_start(
                out=tW4_all[0:4, :],
                in_=bass.AP(tensor=twd.tensor, offset=twd.offset,
                            ap=[[512, 4], [2048, N // 4], [1, 512]]),
            )
            for s in range(3):
                nc.sync.dma_start(
                    out=tW_f[ds(32 * s, 1), :],
                    in_=tW_dram[br, ds(s * 32 * D, 32 * D)],
                )

            # running sum over n of relu(h2) (4 lanes folded at the end)
            macc3 = work.tile([128, 2, 384], F32, tag="macc3")
            nc.vector.memset(macc3[:], 0.0)

            # ---- main loop: 32 triads of n = {g, 32+g, 64+g} ---------------
            for g in range(32):
                h1_s = hpool.tile([128, 4, 384], F32R, tag="h1")
                for c in range(4):
                    ph1 = psH1.tile([128, 384], F32, tag="h1p")
                    # broadcast aT (+b1) into PSUM via f32r identity matmul
                    nc.tensor.matmul(
                        ph1[:], I_r[:], aT3_s[:, c, :],
                        start=True, stop=True,
                    )
                    # rank-1 updates: + t[n, c-chunk] (x) e[:, n], per triad slice
                    for s in range(3):
                        nc.tensor.matmul(
                            ph1[:, ts(s, 128)],
                            tW_f[ds(32 * s, 1), ds(g * D + c * 128, 128)],
                            eT_f[ds(32 * s, 1), ts(g, BL)],
                            start=False, stop=True, skip_group_check=True,
                            tile_position=(32 * s, 0),
                        )
                    # relu -> SBUF (split across ScalarE / VectorE)
                    if c < 3:
                        nc.scalar.activation(h1_s[:, c, :], ph1[:], AF.Relu)
                    else:
                        nc.vector.tensor_scalar_max(h1_s[:, c, :], ph1[:], 0.0)

                for m in range(2):
                    ph2 = psH2.tile([128, 384], F32, tag="h2p")
                    # seed PSUM with b2 broadcast: b2_half (x) ones
                    nc.tensor.matmul(
                        ph2[:],
                        b2row_r[0:1, ts(m, 128)], ones_r[0:1, :],
                        start=True, stop=False,
                    )
                    for c in range(4):
                        nc.tensor.matmul(
                            ph2[:],
                            W2_r[:, c, ts(m, 128)], h1_s[:, c, :],
                            start=False, stop=(c == 3),
                        )
                    # fused: macc3 += relu(ph2)
                    nc.vector.scalar_tensor_tensor(
                        out=macc3[:, m, :], in0=ph2[:], scalar=0.0,
                        in1=macc3[:, m, :], op0=ALU.max, op1=ALU.add,
                    )

            # fold 3 lanes then layer 3
            mfold = work.tile([128, 2, 128], F32, tag="mfold")
            nc.vector.tensor_add(mfold[:], macc3[:, :, 0:128], macc3[:, :, 128:256])
            mred = work.tile([128, 2, 128], F32, tag="mred")
            nc.vector.tensor_add(mred[:], mfold[:], macc3[:, :, 256:384])
            po = psH1.tile([128, 128], F32, tag="h1p")
            for m in range(2):
                nc.tensor.matmul(
                    po[:], W3_s[:, m, :], mred[:, m, :],
                    start=(m == 0), stop=(m == 1),
                )
            outT = work.tile([128, 128], F32, tag="outT")
            nc.vector.tensor_scalar(
                out=outT[:], in0=po[:], scalar1=1.0 / N, scalar2=b3t[:, 0:1],
                op0=ALU.mult, op1=ALU.add,
            )
            nc.sync.dma_start(out=out_ext[br, :, :], in_=outT[:])

    return nc


def make_in_maps(inputs):
    import ml_dtypes

    img = np.ascontiguousarray(np.asarray(inputs["img_embs"], np.float32))
    shared = {
        "tn": np.ascontiguousarray(np.asarray(inputs["normal_text_embs"], np.float32)),
        "ta": np.ascontiguousarray(np.asarray(inputs["abnormal_text_embs"], np.float32)),
        "W1": np.ascontiguousarray(np.asarray(inputs["W1"], np.float32)),
        "b1": np.ascontiguousarray(np.asarray(inputs["b1"], np.float32)),
        "W2": np.ascontiguousarray(np.asarray(inputs["W2"], np.float32)),
        "b2": np.ascontiguousarray(np.asarray(inputs["b2"], np.float32)),
        "W3": np.ascontiguousarray(np.asarray(inputs["W3"], np.float32)),
        "b3": np.ascontiguousarray(np.asarray(inputs["b3"], np.float32)),
        "idf": np.eye(128, dtype=np.float32),
        "idb": np.eye(128).astype(ml_dtypes.bfloat16),
        "zer": np.zeros((N // 4) * 2048, dtype=ml_dtypes.bfloat16),
    }
    return [dict(shared, img=img[i * BL : (i + 1) * BL]) for i in range(NC)]


def kernel(**inputs) -> tuple:
    _install_bir_patch()

    b2 = np.asarray(inputs["b2"], np.float32)
    scale = float(np.exp(np.asarray(inputs["logit_scale"], np.float32).reshape(-1)[0]))

    nc = build_graph(scale, has_b2=bool(np.any(b2)))
    in_maps = make_in_maps(inputs)
    res = run_bass_kernel_spmd(nc, in_maps, core_ids=list(range(NC)))
    h_n = np.concatenate([res.results[i]["out"][0].T for i in range(NC)], axis=0)
    h_a = np.concatenate([res.results[i]["out"][1].T for i in range(NC)], axis=0)
    return (h_n, h_a)
